# revision 1
# baseline (speedup 1.0000x reference)
"""Exact attention (B=2, N=2048, H=16, D=64, fp32) on 8 Trainium2 NeuronCores.

Sharding: the 32 (batch, head) pairs are split across 8 cores, 4 heads per
core. Each core computes full (non-causal, unscaled) attention for its heads.

Per-core kernel layout (heads processed as 2 head-pairs):
  - Q, K are PE-transposed into [d, n] layout, with head-pair packing: one
    [128, 128] transpose yields head0's d-rows on partitions 0-63 and head1's
    on partitions 64-127.
  - Main loop over (n-half of 1024) x (m-block of 64):
      S^T[m, n] = K Q^T via two concurrent quadrant matmuls (f32r, full rate)
      P^T = exp(S^T) on the ACT engine - one [128, 1024] instruction covers
      both heads (ACT is the roofline: N^2 exps at 1 elem/cycle/lane).
      O^T[65, n] += V'^T P^T where V' = [V | ones]; col 64 accumulates the
      softmax denominators. Two concurrent row-group matmuls (f32r).
  - Finalize: flush O^T to SBUF, PE-transpose 128-col chunks to [n, 65],
    reciprocal of col 64, tensor_scalar multiply, DMA out.

Numerics: matmuls use float32r (fp32 with 11-bit mantissa, full PE rate).
"""

import os
import sys

os.environ.setdefault("MYCRO_LOCAL_CACHE", "1")
sys.path.insert(0, "/opt/trn_rl_repo")

import numpy as np

import concourse.bacc as bacc
import concourse.mybir as mybir
import concourse.tile as tile
from concourse.bass_utils import run_bass_kernel_spmd
from concourse.masks import make_identity

f32 = mybir.dt.float32
f32r = mybir.dt.float32r

B, N, H, D = 2, 2048, 16, 64
HEADS_PER_CORE = 4
N_CORES = 8
NH = 1024          # n-half width
N_MB = N // 128    # 16 m-blocks of 128 rows
DV = D + 1         # V plus ones column


def emit_body(nc, q, k, v, out, pools):
    """Emit one full attention pass for 4 heads ([4, N, D] DRAM tensors).

    The ACT engine (softmax exp, 1 elem/cycle/lane) is the roofline; the
    schedule keeps exp tiles at [128, 1024] full-lane and PE/DVE/DMA work
    sized to fit underneath it.
    """
    const, stage, qkt, vt_p, spool, ppool, opool, otflush, finsb = pools
    identity = const["identity"]

    def emit_stage_dmas(pair):
        """Issue staging + V DMAs for a pair; returns (qt, kt, vts, sgs)."""
        h0, h1 = 2 * pair, 2 * pair + 1
        qt = qkt.tile([128, N], f32r, name=f"qt_{pair}", tag="qt")
        kt = qkt.tile([128, N], f32r, name=f"kt_{pair}", tag="kt")
        sgs = {}
        for src, nm in ((q, "q"), (k, "k")):
            # [128, 16, 128] staging: tile t holds rows t*128.. of both heads
            # (h0 in cols 0:64, h1 in 64:128), in 4-tile DMA chunks.
            sg = stage.tile([128, N // 128, 128], f32,
                            name=f"sg_{nm}_{pair}", tag=f"sg_{nm}")
            for g in range(4):
                gt = slice(g * 4, (g + 1) * 4)
                gr = slice(g * 512, (g + 1) * 512)
                nc.sync.dma_start(
                    out=sg[:, gt, 0:64],
                    in_=src[h0, gr, :].rearrange("(t p) d -> p t d", p=128))
                nc.sync.dma_start(
                    out=sg[:, gt, 64:128],
                    in_=src[h1, gr, :].rearrange("(t p) d -> p t d", p=128))
            sgs[nm] = sg
        vts = []
        for hh in (h0, h1):
            vt = vt_p.tile([128, N_MB, DV], f32r, name=f"vt_{hh}", tag=f"vt{hh % 2}")
            nc.sync.dma_start(
                out=vt[:, :, 0:64],
                in_=v.bitcast(f32r)[hh].rearrange("(mb p) d -> p mb d", p=128),
            )
            nc.vector.tensor_copy(vt[:, :, 64:65], const["ones"])
            vts.append(vt)
        return qt, kt, vts, sgs

    def transpose_task(pair, sg, dst, nm, t):
        def go():
            tp = spool.tile([128, 128], f32, name=f"tp_{nm}_{pair}_{t}", tag="s")
            nc.tensor.transpose(tp, sg[:, t, :], identity)
            # rounding producer: fp32 psum -> f32r sbuf (DVE only: any op
            # queued on the ACT sequencer can head-of-line block exps)
            nc.vector.tensor_copy(dst[:, t * 128:(t + 1) * 128], tp)
        return go

    def fin_task(pair, nh, hh, ots, ostage, c):
        def go():
            csl = slice(c * 128, (c + 1) * 128)
            fin = spool.tile([128, 65], f32,
                             name=f"fin_{pair}_{nh}_{hh}_{c}", tag="s")
            nc.tensor.transpose(fin, ots[:, csl], identity[0:65, 0:65])
            rcp = finsb.tile([128, 1], f32,
                             name=f"rcp_{pair}_{nh}_{hh}_{c}", tag="rcp")
            nc.vector.reciprocal(rcp, fin[:, 64:65])
            nc.vector.tensor_scalar_mul(ostage[:, c, :], fin[:, 0:64], rcp)
            if c == NH // 128 - 1:
                nc.sync.dma_start(
                    out=out[hh].rearrange("(cc p) d -> p cc d", p=128)[
                        :, nh * (NH // 128):(nh + 1) * (NH // 128), :],
                    in_=ostage)
        return go

    # stage pair 0, transpose it, then issue pair 1's staging DMAs right
    # away: they run on the otherwise-idle DMA engines during pair 0's
    # compute, so pair 1's prep no longer waits on HBM.
    state = [emit_stage_dmas(0)]
    for t in range(N // 128):
        transpose_task(0, state[0][3]["q"], state[0][0], "q", t)()
        transpose_task(0, state[0][3]["k"], state[0][1], "k", t)()
    state.append(emit_stage_dmas(1))

    for pair in range(2):
        qt, kt, vts, sgs = state[pair]
        if pair == 1:
            for t in range(N // 128):
                transpose_task(1, sgs["q"], qt, "q", t)()
                transpose_task(1, sgs["k"], kt, "k", t)()
        h0, h1 = 2 * pair, 2 * pair + 1

        for nh in range(N // NH):
            oaccs = [
                opool.tile([65, NH], f32, name=f"o_{pair}_{nh}_{i}", tag=f"o{i}")
                for i in range(2)
            ]
            for mb in range(N_MB):
                msl = slice(mb * 128, (mb + 1) * 128)
                first, last = mb == 0, mb == N_MB - 1
                pts = []
                for i, plo in ((0, 0), (1, 64)):
                    sp = spool.tile([128, NH], f32,
                                    name=f"sp_{pair}_{nh}_{mb}_{i}", tag="s")
                    for j in range(NH // 512):
                        jsl = slice(nh * NH + j * 512, nh * NH + (j + 1) * 512)
                        osl = slice(j * 512, (j + 1) * 512)
                        nc.tensor.matmul(
                            out=sp[:, osl], lhsT=kt[plo:plo + 64, msl],
                            rhs=qt[plo:plo + 64, jsl], start=True, stop=True)
                    pt = ppool.tile([128, NH], f32r,
                                    name=f"pt_{pair}_{nh}_{mb}_{i}", tag="p")
                    nc.scalar.activation(
                        out=pt, in_=sp, func=mybir.ActivationFunctionType.Exp)
                    pts.append(pt)
                for i in range(2):
                    for j in range(NH // 512):
                        osl = slice(j * 512, (j + 1) * 512)
                        nc.tensor.matmul(
                            out=oaccs[i][:, osl], lhsT=vts[i][:, mb, :],
                            rhs=pts[i][:, osl], start=first, stop=last)

            # flush O^T accumulators (frees the opool banks; split each copy
            # across DVE and ACT so the new n-half's O-matmuls unblock fast),
            # then run this n-half's finalize chunks
            for hh, ot in ((h0, oaccs[0]), (h1, oaccs[1])):
                ots = otflush.tile([65, NH], f32,
                                   name=f"ots_{pair}_{nh}_{hh}", tag="ots")
                nc.vector.tensor_copy(ots, ot)
                ostage = finsb.tile([128, NH // 128, 64], f32,
                                    name=f"ostage_{pair}_{nh}_{hh}", tag="ostage")
                for c in range(NH // 128):
                    fin_task(pair, nh, hh, ots, ostage, c)()


def build(repeat=1):
    nc = bacc.Bacc("TRN2", target_bir_lowering=False, debug=False)
    q = nc.dram_tensor("q", [HEADS_PER_CORE, N, D], f32, kind="ExternalInput").ap()
    k = nc.dram_tensor("k", [HEADS_PER_CORE, N, D], f32, kind="ExternalInput").ap()
    v = nc.dram_tensor("v", [HEADS_PER_CORE, N, D], f32, kind="ExternalInput").ap()
    out = nc.dram_tensor("out", [HEADS_PER_CORE, N, D], f32, kind="ExternalOutput").ap()

    from contextlib import ExitStack
    with tile.TileContext(nc) as tc, ExitStack() as ctx:
        const_pool = ctx.enter_context(tc.tile_pool(name="const", bufs=1))
        identity = const_pool.tile([128, 128], f32, name="identity")
        make_identity(nc, identity)
        ones = const_pool.tile([128, N_MB, 1], f32, name="ones")
        nc.vector.memset(ones, 1.0)

        stage = ctx.enter_context(tc.tile_pool(name="stage", bufs=2))
        qkt = ctx.enter_context(tc.tile_pool(name="qkt", bufs=2))
        vt_p = ctx.enter_context(tc.tile_pool(name="vt", bufs=2))
        # transposes + S tiles share one psum pool (tag "s"): 2 bufs x 2 banks
        spool = ctx.enter_context(tc.tile_pool(name="spool", bufs=2, space="PSUM"))
        ppool = ctx.enter_context(tc.tile_pool(name="ppool", bufs=3))
        opool = ctx.enter_context(tc.tile_pool(name="opool", bufs=1, space="PSUM"))
        otflush = ctx.enter_context(tc.tile_pool(name="otflush", bufs=4))
        finsb = ctx.enter_context(tc.tile_pool(name="finsb", bufs=4))

        pools = ({"identity": identity, "ones": ones}, stage, qkt, vt_p, spool,
                 ppool, opool, otflush, finsb)

        if repeat == 1:
            emit_body(nc, q, k, v, out, pools)
        else:
            # hint_engines: the body far exceeds one IRAM block per engine,
            # so arm the back-edge branch prefetch to avoid ~4us I$-miss
            # stalls per iteration in the timing loop.
            with tc.For_i(0, repeat, 1, hint_engines=(
                    mybir.EngineType.PE, mybir.EngineType.Activation,
                    mybir.EngineType.DVE, mybir.EngineType.SP)):
                emit_body(nc, q, k, v, out, pools)

    nc.compile()
    return nc


_NC_CACHE = {}


def _get_nc(repeat=1):
    if repeat not in _NC_CACHE:
        _NC_CACHE[repeat] = build(repeat)
    return _NC_CACHE[repeat]


def run_sharded(query, key, value, repeat=1, **spmd_kwargs):
    """query/key/value: [B, N, H, D] fp32 -> out [B, H, N, D] fp32."""
    nc = _get_nc(repeat)
    # [B, N, H, D] -> [B*H, N, D]
    qh = np.ascontiguousarray(np.transpose(query, (0, 2, 1, 3))).reshape(B * H, N, D)
    kh = np.ascontiguousarray(np.transpose(key, (0, 2, 1, 3))).reshape(B * H, N, D)
    vh = np.ascontiguousarray(np.transpose(value, (0, 2, 1, 3))).reshape(B * H, N, D)
    in_maps = [
        {
            "q": qh[c * HEADS_PER_CORE:(c + 1) * HEADS_PER_CORE],
            "k": kh[c * HEADS_PER_CORE:(c + 1) * HEADS_PER_CORE],
            "v": vh[c * HEADS_PER_CORE:(c + 1) * HEADS_PER_CORE],
        }
        for c in range(N_CORES)
    ]
    res = run_bass_kernel_spmd(nc, in_maps, core_ids=list(range(N_CORES)),
                               **spmd_kwargs)
    outs = np.stack([res.results[c]["out"] for c in range(N_CORES)])  # [8, 4, N, D]
    return outs.reshape(B, H, N, D)


def kernel(query, key, value):
    query = np.asarray(query, dtype=np.float32)
    key = np.asarray(key, dtype=np.float32)
    value = np.asarray(value, dtype=np.float32)
    return run_sharded(query, key, value)


if __name__ == "__main__":
    rng = np.random.default_rng(0)
    q = rng.standard_normal((B, N, H, D), dtype=np.float32)
    k = rng.standard_normal((B, N, H, D), dtype=np.float32)
    v = rng.standard_normal((B, N, H, D), dtype=np.float32)
    o = kernel(q, k, v)
    print("out shape:", o.shape, o.dtype)



# revision 6
# speedup vs baseline: 1.9407x; 1.9407x over previous
"""Exact attention (B=2, N=2048, H=16, D=64, fp32) on 8 Trainium2 NeuronCores.

Sharding: the 32 (batch, head) pairs are split across 8 cores, 4 heads per
core. Each core computes full (non-causal, unscaled) attention for its heads.

Per-core schedule: an ACT-bound software pipeline. The exp stream (128
instructions of [128, 1024], ~0.98us each, ~125us total) is the hard floor;
everything else is scheduled to hide underneath it.

  - 64 "slots", one per (pair, n-half, m-block): slot s emits
      S^T(s) on PE   (4 f32r matmuls [64x128 stationary, 64x512 moving])
      exp(s) on ACT  (one [128, 1024] PSUM->SBUF instruction, bf16 out)
      O^T(s-2) on PE (2 bf16 matmuls [128x65 stationary, 128x1024 moving])
    The 2-slot O deferral keeps PE's in-order queue from ever waiting on
    ACT: by the time O(s-2) issues, exp(s-2) finished a slot ago.
  - Q/K are PE-transposed to [d, n] head-pair-packed layout in batches of 8
    tiles through a PSUM staging tile + one [128, 1024] DVE copy. Pair-0
    batches borrow the O-accumulator PSUM banks before the first O matmul;
    pair-1 batches trickle through the S-tile PSUM pool mid-phase.
  - Finalize per (pair, nh): DVE flushes O^T [65,1024] to SBUF, PE
    transposes 128-col chunks into a [128, 8, 128] PSUM batch (reusing the
    just-freed O banks), DVE reciprocal+scale, DMA out. Interleaved with the
    next phase's slots.

Numerics: S matmuls in float32r (11-bit mantissa, full PE rate); P = exp(S)
stored bf16 (P in [0, e^~45], rel err ~0.2%); O accumulated in fp32 PSUM.
"""

import os
import sys

os.environ.setdefault("MYCRO_LOCAL_CACHE", "1")
sys.path.insert(0, "/opt/trn_rl_repo")

import numpy as np

import concourse.bacc as bacc
import concourse.mybir as mybir
import concourse.tile as tile
from concourse.bass_utils import run_bass_kernel_spmd
from concourse.masks import make_identity

f32 = mybir.dt.float32
f32r = mybir.dt.float32r
bf16 = mybir.dt.bfloat16

B, N, H, D = 2, 2048, 16, 64
HEADS_PER_CORE = 4
N_CORES = 8
NH = 1024          # n-half width (exp tile free dim)
N_MB = N // 128    # 16 m-blocks of 128 keys
DV = D + 1         # V plus ones column
N_SLOT = 64        # 2 pairs x 2 n-halves x 16 m-blocks
DEFER = 2          # O matmuls run this many slots behind S/exp


def emit_body(nc, q, k, v, out, pools):
    (const, sgp, vsp, qkt, vt_p, spool, ppool, opool, otflush, finsb) = pools
    identity = const["identity"]
    ones = const["ones"]

    state = {}  # per-pair staging/persistent tiles

    def stage_pair(pair):
        """Issue staging DMAs for a pair's Q, K, V (K chunks first)."""
        h0, h1 = 2 * pair, 2 * pair + 1
        st = {}
        for src, nm in ((k, "k"), (q, "q")):
            sg = sgp.tile([128, N // 128, 128], f32,
                          name=f"sg_{nm}_{pair}", tag=f"sg_{nm}")
            for g in range(4):
                gt = slice(g * 4, (g + 1) * 4)
                gr = slice(g * 512, (g + 1) * 512)
                nc.sync.dma_start(
                    out=sg[:, gt, 0:64],
                    in_=src[h0, gr, :].rearrange("(t p) d -> p t d", p=128))
                nc.sync.dma_start(
                    out=sg[:, gt, 64:128],
                    in_=src[h1, gr, :].rearrange("(t p) d -> p t d", p=128))
            st[nm] = sg
        st["v"] = []
        for i, hh in enumerate((h0, h1)):
            vs = vsp.tile([128, N_MB, 64], f32, name=f"vs_{hh}", tag=f"vs{i}")
            nc.sync.dma_start(
                out=vs, in_=v[hh].rearrange("(mb p) d -> p mb d", p=128))
            st["v"].append(vs)
        qt = qkt.tile([128, N], f32r, name=f"qt_{pair}", tag="qt")
        kt = qkt.tile([128, N], f32r, name=f"kt_{pair}", tag="kt")
        st["qt"], st["kt"] = qt, kt
        state[pair] = st

    def vt_convert(pair):
        """V f32 staging -> bf16 [128, mb, 65] with ones column."""
        vts = []
        for i in range(2):
            vt = vt_p.tile([128, N_MB, DV], bf16,
                           name=f"vt_{pair}_{i}", tag=f"vt{2 * pair + i}")
            nc.vector.tensor_copy(vt[:, :, 0:64], state[pair]["v"][i])
            nc.vector.tensor_copy(vt[:, :, 64:65], ones)
            vts.append(vt)
        state[pair]["vt"] = vts

    def t_batch(pair, nm, b, pool, tag):
        """Transpose 8 staged [128,128] tiles into qt/kt cols b*1024..+1024.

        One PSUM batch tile + one [128, 1024] DVE copy (f32 -> f32r round).
        """
        sg = state[pair][nm]
        dst = state[pair]["qt" if nm == "q" else "kt"]
        tb = pool.tile([128, 8, 128], f32, name=f"tb_{pair}_{nm}_{b}", tag=tag)
        for idx in range(8):
            t = b * 8 + idx
            nc.tensor.transpose(tb[:, idx, :], sg[:, t, :], identity)
        nc.vector.tensor_copy(
            dst[:, b * 1024:(b + 1) * 1024],
            tb.rearrange("p a b -> p (a b)"))

    def emit_S(pair, nh, mb, s):
        qt, kt = state[pair]["qt"], state[pair]["kt"]
        msl = slice(mb * 128, (mb + 1) * 128)
        sps = []
        for i, plo in ((0, 0), (1, 64)):
            sp = spool.tile([128, NH], f32, name=f"sp_{s}_{i}", tag="s")
            for j in range(2):
                jsl = slice(nh * NH + j * 512, nh * NH + (j + 1) * 512)
                nc.tensor.matmul(
                    out=sp[:, j * 512:(j + 1) * 512],
                    lhsT=kt[plo:plo + 64, msl],
                    rhs=qt[plo:plo + 64, jsl], start=True, stop=True)
            sps.append(sp)
        return sps

    def emit_exp(sps, s):
        pts = []
        for i in range(2):
            pt = ppool.tile([128, NH], bf16, name=f"pt_{s}_{i}", tag="p")
            nc.scalar.activation(
                out=pt, in_=sps[i], func=mybir.ActivationFunctionType.Exp)
            pts.append(pt)
        return pts

    def emit_O(pair, nh, mb, pts, oaccs):
        # j-sliced so each matmul's PSUM write stays inside one bank
        for i in range(2):
            for j in range(2):
                jsl = slice(j * 512, (j + 1) * 512)
                nc.tensor.matmul(
                    out=oaccs[i][:, jsl], lhsT=state[pair]["vt"][i][:, mb, :],
                    rhs=pts[i][:, jsl], start=(mb == 0), stop=(mb == N_MB - 1))

    def emit_flush(phase, oaccs):
        pair, nh = divmod(phase, 2)
        otss = []
        for i in range(2):
            ots = otflush.tile([65, NH], f32,
                               name=f"ots_{phase}_{i}", tag=f"ots{i}")
            nc.vector.tensor_copy(ots, oaccs[i])
            otss.append(ots)
        return otss

    def emit_fins(phase, otss):
        """Transpose + normalize + DMA out for one (pair, nh)."""
        pair, nh = divmod(phase, 2)
        for i in range(2):
            hh = 2 * pair + i
            fb = opool.tile([128, 8, 128], f32,
                            name=f"fb_{phase}_{i}", tag=f"o{i}")
            for c in range(8):
                nc.tensor.transpose(
                    fb[:, c, 0:65], otss[i][:, c * 128:(c + 1) * 128],
                    identity[0:65, 0:65])
            ostage = finsb.tile([128, 8, 64], f32,
                                name=f"ostage_{phase}_{i}", tag=f"ostage{i}")
            for c in range(8):
                rcp = finsb.tile([128, 1], f32,
                                 name=f"rcp_{phase}_{i}_{c}", tag=f"rcp{i}")
                nc.vector.reciprocal(rcp, fb[:, c, 64:65])
                nc.vector.tensor_scalar_mul(ostage[:, c, :], fb[:, c, 0:64], rcp)
            nc.sync.dma_start(
                out=out[hh].rearrange("(cc p) d -> p cc d", p=128)[
                    :, nh * 8:(nh + 1) * 8, :],
                in_=ostage)

    # ---- schedule ----
    stage_pair(0)
    # pair-0 transpose batches borrow the o-tags before the first O alloc
    t_batch(0, "k", 0, opool, "o0")
    t_batch(0, "q", 0, opool, "o1")
    t_batch(0, "k", 1, opool, "o0")
    t_batch(0, "q", 1, opool, "o1")
    vt_convert(0)

    # pair-1 transpose batches trickle through spool mid-phase
    p1_tjobs = {12: ("k", 0), 14: ("q", 0), 33: ("k", 1), 35: ("q", 1)}

    pend = {}          # slot -> (sps tiles awaiting exp->O)
    oaccs_by_phase = {}
    otss_by_phase = {}

    for s in range(N_SLOT + DEFER):
        if s < N_SLOT:
            phase, mb = divmod(s, 16)
            pair, nh = divmod(phase, 2)
            if s == 8:
                stage_pair(1)
            if s == 30:
                vt_convert(1)
            if s in p1_tjobs:
                nm, b = p1_tjobs[s]
                t_batch(1, nm, b, spool, "s")
            sps = emit_S(pair, nh, mb, s)
            pend[s] = emit_exp(sps, s)
        if s >= DEFER:
            s2 = s - DEFER
            phase2, mb2 = divmod(s2, 16)
            pair2, nh2 = divmod(phase2, 2)
            if mb2 == 0:
                oaccs_by_phase[phase2] = [
                    opool.tile([65, NH], f32, name=f"o_{phase2}_{i}",
                               tag=f"o{i}")
                    for i in range(2)
                ]
            emit_O(pair2, nh2, mb2, pend.pop(s2), oaccs_by_phase[phase2])
            if mb2 == N_MB - 1:
                otss_by_phase[phase2] = emit_flush(
                    phase2, oaccs_by_phase.pop(phase2))
        # fins for phase P right after its flush (slot P*16+18)
        if s >= 18 and (s - 18) % 16 == 0:
            emit_fins((s - 18) // 16, otss_by_phase.pop((s - 18) // 16))
    emit_fins(3, otss_by_phase.pop(3))


def build(repeat=1):
    nc = bacc.Bacc("TRN2", target_bir_lowering=False, debug=False)
    q = nc.dram_tensor("q", [HEADS_PER_CORE, N, D], f32, kind="ExternalInput").ap()
    k = nc.dram_tensor("k", [HEADS_PER_CORE, N, D], f32, kind="ExternalInput").ap()
    v = nc.dram_tensor("v", [HEADS_PER_CORE, N, D], f32, kind="ExternalInput").ap()
    out = nc.dram_tensor("out", [HEADS_PER_CORE, N, D], f32, kind="ExternalOutput").ap()

    from contextlib import ExitStack
    with tile.TileContext(nc) as tc, ExitStack() as ctx:
        const_pool = ctx.enter_context(tc.tile_pool(name="const", bufs=1))
        identity = const_pool.tile([128, 128], f32, name="identity")
        make_identity(nc, identity)
        ones = const_pool.tile([128, N_MB, 1], f32, name="ones")
        nc.vector.memset(ones, 1.0)

        sgp = ctx.enter_context(tc.tile_pool(name="sgp", bufs=2))
        vsp = ctx.enter_context(tc.tile_pool(name="vsp", bufs=2))
        qkt = ctx.enter_context(tc.tile_pool(name="qkt", bufs=2))
        vt_p = ctx.enter_context(tc.tile_pool(name="vt", bufs=1))
        spool = ctx.enter_context(tc.tile_pool(name="spool", bufs=2, space="PSUM"))
        ppool = ctx.enter_context(tc.tile_pool(name="ppool", bufs=8))
        opool = ctx.enter_context(tc.tile_pool(name="opool", bufs=1, space="PSUM"))
        otflush = ctx.enter_context(tc.tile_pool(name="otflush", bufs=2))
        finsb = ctx.enter_context(tc.tile_pool(name="finsb", bufs=2))

        pools = ({"identity": identity, "ones": ones}, sgp, vsp, qkt, vt_p,
                 spool, ppool, opool, otflush, finsb)

        if repeat == 1:
            emit_body(nc, q, k, v, out, pools)
        else:
            # hint_engines: the body far exceeds one IRAM block per engine,
            # so arm the back-edge branch prefetch to avoid ~4us I$-miss
            # stalls per iteration in the timing loop.
            with tc.For_i(0, repeat, 1, hint_engines=(
                    mybir.EngineType.PE, mybir.EngineType.Activation,
                    mybir.EngineType.DVE, mybir.EngineType.SP)):
                emit_body(nc, q, k, v, out, pools)

    nc.compile()
    return nc


_NC_CACHE = {}


def _get_nc(repeat=1):
    if repeat not in _NC_CACHE:
        _NC_CACHE[repeat] = build(repeat)
    return _NC_CACHE[repeat]


def run_sharded(query, key, value, repeat=1, **spmd_kwargs):
    """query/key/value: [B, N, H, D] fp32 -> out [B, H, N, D] fp32."""
    nc = _get_nc(repeat)
    # [B, N, H, D] -> [B*H, N, D]
    qh = np.ascontiguousarray(np.transpose(query, (0, 2, 1, 3))).reshape(B * H, N, D)
    kh = np.ascontiguousarray(np.transpose(key, (0, 2, 1, 3))).reshape(B * H, N, D)
    vh = np.ascontiguousarray(np.transpose(value, (0, 2, 1, 3))).reshape(B * H, N, D)
    in_maps = [
        {
            "q": qh[c * HEADS_PER_CORE:(c + 1) * HEADS_PER_CORE],
            "k": kh[c * HEADS_PER_CORE:(c + 1) * HEADS_PER_CORE],
            "v": vh[c * HEADS_PER_CORE:(c + 1) * HEADS_PER_CORE],
        }
        for c in range(N_CORES)
    ]
    res = run_bass_kernel_spmd(nc, in_maps, core_ids=list(range(N_CORES)),
                               **spmd_kwargs)
    outs = np.stack([res.results[c]["out"] for c in range(N_CORES)])  # [8, 4, N, D]
    return outs.reshape(B, H, N, D)


def kernel(query, key, value):
    query = np.asarray(query, dtype=np.float32)
    key = np.asarray(key, dtype=np.float32)
    value = np.asarray(value, dtype=np.float32)
    return run_sharded(query, key, value)


if __name__ == "__main__":
    rng = np.random.default_rng(0)
    q = rng.standard_normal((B, N, H, D), dtype=np.float32)
    k = rng.standard_normal((B, N, H, D), dtype=np.float32)
    v = rng.standard_normal((B, N, H, D), dtype=np.float32)
    o = kernel(q, k, v)
    print("out shape:", o.shape, o.dtype)
